# revision 17
# baseline (speedup 1.0000x reference)
"""NTM addressing head (nn_HeadBase) Trainium2 Bass kernel, v2.

Full-input contract: kernel(**inputs) takes the unsharded [256, ...] arrays,
shards batch-dim across 8 NeuronCores (pure data parallel), runs one SPMD Bass
program per core, and gathers the full [256, 4096] output.

v2 engine assignment (per-core: B=32 batches, N=4096, M=64, layout
n = p*32 + r on 128 partitions, streamed in 8 chunks of 4 batches):
  - dot(k, mem) via a CUSTOM DVE op: fused multiply + inclusive prefix-scan
    (scan(ADD, Src0*Src1)) in ONE DVE pass; per-row dots recovered by
    differencing the prefix at 64-element window ends (2 tiny DVE ops).
  - ssq(mem) via ACT square (in-place) + GpSimd tree-fold (64->32->16 strided
    tensor_tensor adds) + DVE reduce of the final 16.
  - phase B (softmax/interp/shift/sharpen) runs PER CHUNK so it overlaps the
    stream; +-1 partition carries for the circular shift use two tiny
    TensorEngine matmuls against constant rotation matrices (extra inputs)
    instead of elementwise SBUF-SBUF DMAs.
"""

import numpy as np

B_FULL, N, M = 256, 4096, 64
NCORES = 8
B = B_FULL // NCORES   # 32 batches per core
P = 128                # SBUF partitions
R = N // P             # 32 rows per partition; n = p*R + r
CB = 16                # batches per phase-B block (phase A is per-batch)
NCHUNK = B // CB       # 2 blocks

_NC_CACHE = {}

_SCAN_OP_NAME = "PRODUCT_PREFIX_SCAN_ANT"


def _register_scan_op():
    """Register a custom DVE op: out = inclusive prefix sum of in0*in1 along
    the free stream. Uses the documented runtime extension points of
    concourse.dve_ops (OPS / CUSTOM_DVE_SPECS / row map)."""
    import concourse.dve_ops as dve_ops
    from concourse.dve_ops import DveOp, _CUSTOM_DVE_ROW_BASE
    from concourse.dve_spec import AluOp, Spec, Src0, Src1, scan, lower, _has_src1
    from concourse.dve_uop import DveOpSpec

    for op in dve_ops.OPS:
        if op.name == _SCAN_OP_NAME:
            return op

    def _ref(in0, in1, s0, s1, imm2):
        prod = (in0.astype(np.float32) * in1.astype(np.float32)).reshape(
            in0.shape[0], -1
        )
        return np.cumsum(prod, axis=1).astype(np.float32).reshape(in0.shape)

    spec = Spec(body=scan(AluOp.ADD, Src0 * Src1), reference=_ref)
    row = _CUSTOM_DVE_ROW_BASE + len(dve_ops.OPS)
    assert row < 0x20
    shas = {}
    for ver in ("v3", "v4"):
        s = DveOpSpec(
            name=_SCAN_OP_NAME, opcode=row, uops=lower(spec, ver=ver),
            rd1_en=_has_src1(spec),
        )
        shas[ver] = s.sha(ver)
    op = DveOp(_SCAN_OP_NAME, spec, subdim=False, uops_sha=shas)
    dve_ops.OPS.append(op)
    dve_ops.CUSTOM_DVE_SPECS[_SCAN_OP_NAME] = spec
    dve_ops._SUB_OPCODE_FOR_NAME[_SCAN_OP_NAME] = row
    return op


def _build_body(nc, out_ap, ins, scan_op):
    from contextlib import ExitStack

    import concourse.bass as bass
    import concourse.tile as tile
    from concourse import mybir

    f32 = mybir.dt.float32
    Alu = mybir.AluOpType
    Act = mybir.ActivationFunctionType
    Ax = mybir.AxisListType
    AP = bass.AP

    mem_ap = ins["memory"]   # [B, N, M]
    k_ap = ins["k"]          # [B, M]
    beta_ap = ins["beta"]    # [B, 1]
    pw_ap = ins["prev_w"]    # [B, N]
    g_ap = ins["g"]          # [B, 1]
    s_ap = ins["s"]          # [B, 3]
    gam_ap = ins["gamma"]    # [B, 1]
    rotdn_ap = ins["rotdn"]  # [P, P] constant
    rotup_ap = ins["rotup"]  # [P, P] constant

    def bcast_inner(ap2d, n):
        return AP(ap2d.tensor, ap2d.offset, list(ap2d.ap) + [[0, n]])

    def row1(ap1d):
        return AP(ap1d.tensor, ap1d.offset, [[0, 1]] + list(ap1d.ap))

    with tile.TileContext(nc) as tc, ExitStack() as ctx:
        singles = ctx.enter_context(tc.tile_pool(name="singles", bufs=1))
        mem_pool = ctx.enter_context(tc.tile_pool(name="mem", bufs=8))
        pre_pool = ctx.enter_context(tc.tile_pool(name="pre", bufs=4))
        h_pool = ctx.enter_context(tc.tile_pool(name="hp", bufs=4))
        sm = ctx.enter_context(tc.tile_pool(name="sm", bufs=3))
        ps = ctx.enter_context(tc.tile_pool(name="ps", bufs=2, space="PSUM"))
        ps_big = ctx.enter_context(tc.tile_pool(name="psbig", bufs=1, space="PSUM"))

        # ---- setup: constants, per-batch scalar rows on partition 0 ----
        ones_col = singles.tile([P, 1], f32, tag="ones_col")
        nc.vector.memset(ones_col, 1.0)
        ones_row = singles.tile([1, P], f32, tag="ones_row")
        nc.vector.memset(ones_row, 1.0)

        rotdn_t = singles.tile([P, P], f32, tag="rotdn")
        nc.sync.dma_start(out=rotdn_t, in_=rotdn_ap)
        rotup_t = singles.tile([P, P], f32, tag="rotup")
        nc.sync.dma_start(out=rotup_t, in_=rotup_ap)

        k_row = singles.tile([1, B * M], f32, tag="k_row")
        nc.sync.dma_start(out=k_row, in_=row1(k_ap.rearrange("b m -> (b m)")))
        b_row = singles.tile([1, B], f32, tag="b_row")
        nc.sync.dma_start(out=b_row, in_=row1(beta_ap.rearrange("b one -> (b one)")))
        g_row = singles.tile([1, B], f32, tag="g_row")
        nc.sync.dma_start(out=g_row, in_=row1(g_ap.rearrange("b one -> (b one)")))
        gm_row = singles.tile([1, B], f32, tag="gm_row")
        nc.sync.dma_start(out=gm_row, in_=row1(gam_ap.rearrange("b one -> (b one)")))
        s_row = singles.tile([1, 3 * B], f32, tag="s_row")
        nc.sync.dma_start(out=s_row, in_=row1(s_ap.rearrange("b i -> (b i)")))
        s_perm = s_row.rearrange("p (b i) -> p i b", i=3)
        s_v = [s_perm[:, i, :] for i in range(3)]

        # k broadcast to all partitions: kb[p, b*M+m] = k[b, m]
        k_row2 = singles.tile([1, B * M], f32, tag="setup_scratch")
        nc.vector.tensor_copy(k_row2, k_row)
        kb_psum = ps_big.tile([P, B * M], f32, tag="kb_psum")
        for j in range(0, B * M, 512):
            nc.tensor.matmul(
                kb_psum[:, j : j + 512], ones_row, k_row2[:, j : j + 512],
                start=True, stop=True,
            )
        kb = singles.tile([P, B * M], f32, tag="kb")
        nc.scalar.copy(out=kb, in_=kb_psum)

        # knorm; bk = beta / knorm (reuses the setup scratch row)
        ksq_row = singles.tile([1, B * M], f32, tag="setup_scratch")
        nc.vector.tensor_mul(ksq_row, k_row, k_row)
        ks_row = singles.tile([1, B], f32, tag="ks_row")
        nc.vector.tensor_reduce(
            out=ks_row, in_=ksq_row.rearrange("p (b m) -> p b m", m=M),
            axis=Ax.X, op=Alu.add,
        )
        kn_row = singles.tile([1, B], f32, tag="kn_row")
        nc.scalar.activation(out=kn_row, in_=ks_row, func=Act.Sqrt)
        rk_row = singles.tile([1, B], f32, tag="rk_row")
        nc.vector.reciprocal(out=rk_row, in_=kn_row)
        bk_row = singles.tile([1, B], f32, tag="bk_row")
        nc.vector.tensor_mul(bk_row, b_row, rk_row)

        # omg = 1 - g
        omg_row = singles.tile([1, B], f32, tag="omg_row")
        nc.vector.tensor_scalar(
            out=omg_row, in0=g_row, scalar1=-1.0, scalar2=1.0,
            op0=Alu.mult, op1=Alu.add,
        )

        # broadcast: [bk, omg, s0, s1, s2, gamma] -> [P, 6*B]
        NSC = 6
        asm1 = singles.tile([1, NSC * B], f32, tag="asm1")
        for i, src in enumerate([bk_row, omg_row, s_v[0], s_v[1], s_v[2], gm_row]):
            nc.vector.tensor_copy(asm1[:, i * B : (i + 1) * B], src)
        bc1_ps = ps_big.tile([P, NSC * B], f32, tag="bc1_ps")
        nc.tensor.matmul(bc1_ps, ones_row, asm1, start=True, stop=True)
        BC1 = singles.tile([P, NSC * B], f32, tag="BC1")
        nc.scalar.copy(out=BC1, in_=bc1_ps)
        BK = BC1[:, 0 * B : 1 * B]
        OMG = BC1[:, 1 * B : 2 * B]
        S0 = BC1[:, 2 * B : 3 * B]
        S1 = BC1[:, 3 * B : 4 * B]
        S2 = BC1[:, 4 * B : 5 * B]
        GAM = BC1[:, 5 * B : 6 * B]

        # prev_w big tile [P, B*R]
        pw = singles.tile([P, B * R], f32, tag="pw")
        nc.sync.dma_start(
            out=pw.rearrange("p (b r) -> p b r", r=R),
            in_=pw_ap.rearrange("b (p r) -> p b r", r=R),
        )

        # ---- streamed, per-batch pipelined phase A ----
        BW = R * M  # free elems per batch per partition (2048)
        for c in range(NCHUNK):
            b0 = c * CB
            dotw = sm.tile([P, CB * R], f32, tag="dotw")
            dotw3 = dotw.rearrange("p (b r) -> p b r", r=R)
            ssq = sm.tile([P, CB * R], f32, tag="ssq")
            ssq3 = ssq.rearrange("p (b r) -> p b r", r=R)

            for j in range(CB):
                bj = b0 + j
                mt = mem_pool.tile([P, BW], f32, tag="mt")
                nc.sync.dma_start(
                    out=mt,
                    in_=mem_ap[bj].rearrange("(p r) m -> p (r m)", p=P),
                )
                mt3 = mt.rearrange("p (r m) -> p r m", m=M)

                # fused mult+prefix-scan (custom DVE op). The output AP has a
                # 0-stride inner dim: all 64 writes of window r land on
                # ends[p, r]; DVE writes in stream order, so the final value
                # is the prefix at the window end — a compact [P, R] ends
                # tile with no strided re-read.
                pre = pre_pool.tile([P, R], f32, tag="pre")
                kbj = kb[:, bj * M : (bj + 1) * M]
                kb3 = AP(kbj.tensor, kbj.offset, [kbj.ap[0], [0, R], [1, M]])
                nc.vector._custom_dve(
                    scan_op,
                    out=AP(pre.tensor, pre.offset, [pre.ap[0], [1, R], [0, M]]),
                    in0=mt3, in1=kb3,
                )

                # dot rows by differencing consecutive window-end prefixes
                nc.vector.tensor_copy(dotw3[:, j, 0:1], pre[:, 0:1])
                nc.vector.tensor_tensor(
                    out=dotw3[:, j, 1:R], in0=pre[:, 1:R], in1=pre[:, 0 : R - 1],
                    op=Alu.subtract,
                )

                # ssq: ACT square in place, GpSimd fold 64->32->16, DVE reduce
                nc.scalar.square(out=mt, in_=mt)
                hb = h_pool.tile([P, R * 48], f32, tag="hb")
                h1 = hb[:, 0 : R * 32].rearrange("p (r m) -> p r m", m=32)
                nc.gpsimd.tensor_tensor(
                    out=h1, in0=mt3[:, :, 0:32], in1=mt3[:, :, 32:64],
                    op=Alu.add,
                )
                h2 = hb[:, R * 32 : R * 48].rearrange("p (r m) -> p r m", m=16)
                nc.gpsimd.tensor_tensor(
                    out=h2, in0=h1[:, :, 0:16], in1=h1[:, :, 16:32],
                    op=Alu.add,
                )
                nc.vector.tensor_reduce(
                    out=ssq3[:, j, :], in_=h2, axis=Ax.X, op=Alu.add,
                )

            # ---- phase B for this chunk ----
            BKc = BK[:, b0 : b0 + CB]
            OMGc = OMG[:, b0 : b0 + CB]
            S0c = S0[:, b0 : b0 + CB]
            S1c = S1[:, b0 : b0 + CB]
            S2c = S2[:, b0 : b0 + CB]
            GAMc = GAM[:, b0 : b0 + CB]

            def v3(t):
                return t.rearrange("p (b r) -> p b r", r=R)

            # rstd = 1/sqrt(ssq); a = dot * rstd * bk
            mn = sm.tile([P, CB * R], f32, tag="mn")
            nc.scalar.activation(out=mn, in_=ssq, func=Act.Sqrt)
            rstd = sm.tile([P, CB * R], f32, tag="rstd")
            nc.vector.reciprocal_approx_fast(out=rstd, in_=mn)
            nc.vector.tensor_mul(dotw, dotw, rstd)
            nc.vector.tensor_mul(dotw3, dotw3, bcast_inner(BKc, R))

            # e = exp(a); per-batch denom; gd = g/denom
            mmall = ps.tile([P, 6 * CB], f32, tag="mmall")
            den_ps = mmall[0:1, 0 * CB : 1 * CB]
            gd_ps = mmall[:, 1 * CB : 2 * CB]
            dn_ps = mmall[:, 2 * CB : 3 * CB]
            up_ps = mmall[:, 3 * CB : 4 * CB]
            d2_ps = mmall[0:1, 4 * CB : 5 * CB]
            rd2_ps = mmall[:, 5 * CB : 6 * CB]

            e = sm.tile([P, CB * R], f32, tag="e")
            nc.scalar.activation(out=e, in_=dotw, func=Act.Exp)
            cs = sm.tile([P, CB], f32, tag="cs")
            nc.vector.tensor_reduce(out=cs, in_=v3(e), axis=Ax.X, op=Alu.add)
            nc.tensor.matmul(den_ps, ones_col, cs, start=True, stop=True)
            rden = sm.tile([1, CB], f32, tag="rden")
            nc.vector.reciprocal(out=rden, in_=den_ps)
            gd = sm.tile([1, CB], f32, tag="gd")
            nc.vector.tensor_mul(gd, rden, g_row[:, b0 : b0 + CB])
            nc.tensor.matmul(gd_ps, ones_row, gd, start=True, stop=True)

            # wg = e*gd + pw*omg  (GpSimd cannot read PSUM: stage gd in SBUF)
            gd_sb = sm.tile([P, CB], f32, tag="gd_sb")
            nc.scalar.copy(out=gd_sb, in_=gd_ps)
            pwc = sm.tile([P, CB * R], f32, tag="pwc")
            pwv = pw[:, b0 * R : (b0 + CB) * R]
            nc.gpsimd.tensor_tensor(
                out=v3(e), in0=v3(e), in1=bcast_inner(gd_sb, R), op=Alu.mult
            )
            nc.gpsimd.tensor_tensor(
                out=v3(pwc), in0=v3(pwv), in1=bcast_inner(OMGc, R), op=Alu.mult
            )
            nc.gpsimd.tensor_tensor(out=e, in0=e, in1=pwc, op=Alu.add)

            # circular 3-tap shift
            ws = sm.tile([P, CB * R], f32, tag="ws")
            ta = sm.tile([P, CB * R], f32, tag="ta")
            tb = sm.tile([P, CB * R], f32, tag="tb")
            wg3, ws3, ta3, tb3 = v3(e), v3(ws), v3(ta), v3(tb)
            nc.vector.tensor_mul(ws3, wg3, bcast_inner(S1c, R))
            nc.vector.tensor_mul(ta3, wg3, bcast_inner(S0c, R))
            nc.gpsimd.tensor_tensor(
                out=tb3, in0=wg3, in1=bcast_inner(S2c, R), op=Alu.mult
            )
            nc.vector.tensor_add(
                out=ws3[:, :, 1:R], in0=ws3[:, :, 1:R], in1=ta3[:, :, 0 : R - 1]
            )
            nc.vector.tensor_add(
                out=ws3[:, :, 0 : R - 1], in0=ws3[:, :, 0 : R - 1],
                in1=tb3[:, :, 1:R],
            )
            # partition carries via rotation matmuls
            nc.tensor.matmul(
                dn_ps, rotdn_t,
                AP(ta.tensor, ta.offset + R - 1, [ta.ap[0], [R, CB]]),
                start=True, stop=True,
            )
            nc.tensor.matmul(
                up_ps, rotup_t,
                AP(tb.tensor, tb.offset, [tb.ap[0], [R, CB]]),
                start=True, stop=True,
            )
            nc.vector.tensor_add(
                out=ws3[:, :, 0:1], in0=ws3[:, :, 0:1],
                in1=bcast_inner(dn_ps, 1),
            )
            nc.vector.tensor_add(
                out=ws3[:, :, R - 1 : R], in0=ws3[:, :, R - 1 : R],
                in1=bcast_inner(up_ps, 1),
            )

            # w_pow = ws ** gamma; normalize
            nc.scalar.activation(out=ws, in_=ws, func=Act.Ln)
            nc.gpsimd.tensor_tensor(
                out=ws3, in0=ws3, in1=bcast_inner(GAMc, R), op=Alu.mult
            )
            nc.scalar.activation(out=ws, in_=ws, func=Act.Exp)
            cs2 = sm.tile([P, CB], f32, tag="cs2")
            nc.vector.tensor_reduce(out=cs2, in_=ws3, axis=Ax.X, op=Alu.add)
            nc.tensor.matmul(d2_ps, ones_col, cs2, start=True, stop=True)
            d2r = sm.tile([1, CB], f32, tag="d2r")
            nc.vector.tensor_scalar_add(out=d2r, in0=d2_ps, scalar1=1e-16)
            rd2 = sm.tile([1, CB], f32, tag="rd2")
            nc.vector.reciprocal(out=rd2, in_=d2r)
            nc.tensor.matmul(rd2_ps, ones_row, rd2, start=True, stop=True)
            rd2_sb = sm.tile([P, CB], f32, tag="rd2_sb")
            nc.scalar.copy(out=rd2_sb, in_=rd2_ps)
            outsb = sm.tile([P, CB * R], f32, tag="outsb")
            nc.gpsimd.tensor_tensor(
                out=v3(outsb), in0=ws3, in1=bcast_inner(rd2_sb, R), op=Alu.mult
            )

            nc.sync.dma_start(
                out=out_ap[b0 : b0 + CB].rearrange("b (p r) -> p b r", r=R),
                in_=outsb.rearrange("p (b r) -> p b r", r=R),
            )


def _get_nc():
    if "nc" in _NC_CACHE:
        return _NC_CACHE["nc"]
    scan_op = _register_scan_op()
    from concourse import bacc, mybir

    f32 = mybir.dt.float32
    nc = bacc.Bacc("TRN2", debug=False, num_devices=NCORES)
    ins = {
        "memory": nc.dram_tensor("memory", [B, N, M], f32, kind="ExternalInput").ap(),
        "k": nc.dram_tensor("k", [B, M], f32, kind="ExternalInput").ap(),
        "beta": nc.dram_tensor("beta", [B, 1], f32, kind="ExternalInput").ap(),
        "prev_w": nc.dram_tensor("prev_w", [B, N], f32, kind="ExternalInput").ap(),
        "g": nc.dram_tensor("g", [B, 1], f32, kind="ExternalInput").ap(),
        "s": nc.dram_tensor("s", [B, 3], f32, kind="ExternalInput").ap(),
        "gamma": nc.dram_tensor("gamma", [B, 1], f32, kind="ExternalInput").ap(),
        "rotdn": nc.dram_tensor("rotdn", [P, P], f32, kind="ExternalInput").ap(),
        "rotup": nc.dram_tensor("rotup", [P, P], f32, kind="ExternalInput").ap(),
    }
    out_ap = nc.dram_tensor("out", [B, N], f32, kind="ExternalOutput").ap()
    _build_body(nc, out_ap, ins, scan_op)
    nc.finalize()
    _NC_CACHE["nc"] = nc
    return nc


def _rot_mats():
    idx = np.arange(P)
    rotdn = np.zeros((P, P), dtype=np.float32)
    rotdn[(idx - 1) % P, idx] = 1.0   # out[p] = x[(p-1) mod P]
    rotup = np.zeros((P, P), dtype=np.float32)
    rotup[(idx + 1) % P, idx] = 1.0   # out[p] = x[(p+1) mod P]
    return rotdn, rotup


def _shard_inputs(inputs):
    arrs = {
        name: np.ascontiguousarray(np.asarray(inputs[name], dtype=np.float32))
        for name in ("memory", "k", "beta", "prev_w", "g", "s", "gamma")
    }
    rotdn, rotup = _rot_mats()
    in_maps = []
    for c in range(NCORES):
        sl = slice(c * B, (c + 1) * B)
        m = {name: np.ascontiguousarray(a[sl]) for name, a in arrs.items()}
        m["rotdn"] = rotdn
        m["rotup"] = rotup
        in_maps.append(m)
    return in_maps


def run(inputs, trace=False):
    from concourse.bass_utils import run_bass_kernel_spmd

    nc = _get_nc()
    in_maps = _shard_inputs(inputs)
    res = run_bass_kernel_spmd(
        nc, in_maps, core_ids=list(range(NCORES)), trace=trace,
        **({"trace_cores": [0]} if trace else {}),
    )
    out = np.concatenate([r["out"] for r in res.results], axis=0)
    return out, res


def kernel(**inputs):
    out, _ = run(inputs, trace=False)
    return out


# revision 18
# speedup vs baseline: 1.0003x; 1.0003x over previous
"""NTM addressing head (nn_HeadBase) Trainium2 Bass kernel, v2.

Full-input contract: kernel(**inputs) takes the unsharded [256, ...] arrays,
shards batch-dim across 8 NeuronCores (pure data parallel), runs one SPMD Bass
program per core, and gathers the full [256, 4096] output.

v2 engine assignment (per-core: B=32 batches, N=4096, M=64, layout
n = p*32 + r on 128 partitions, streamed in 8 chunks of 4 batches):
  - dot(k, mem) via a CUSTOM DVE op: fused multiply + inclusive prefix-scan
    (scan(ADD, Src0*Src1)) in ONE DVE pass; per-row dots recovered by
    differencing the prefix at 64-element window ends (2 tiny DVE ops).
  - ssq(mem) via ACT square (in-place) + GpSimd tree-fold (64->32->16 strided
    tensor_tensor adds) + DVE reduce of the final 16.
  - phase B (softmax/interp/shift/sharpen) runs PER CHUNK so it overlaps the
    stream; +-1 partition carries for the circular shift use two tiny
    TensorEngine matmuls against constant rotation matrices (extra inputs)
    instead of elementwise SBUF-SBUF DMAs.
"""

import numpy as np

B_FULL, N, M = 256, 4096, 64
NCORES = 8
B = B_FULL // NCORES   # 32 batches per core
P = 128                # SBUF partitions
R = N // P             # 32 rows per partition; n = p*R + r
CB = 8                 # batches per phase-B block (phase A is per-batch)
NCHUNK = B // CB       # 4 blocks

_NC_CACHE = {}

_SCAN_OP_NAME = "PRODUCT_PREFIX_SCAN_ANT"


def _register_scan_op():
    """Register a custom DVE op: out = inclusive prefix sum of in0*in1 along
    the free stream. Uses the documented runtime extension points of
    concourse.dve_ops (OPS / CUSTOM_DVE_SPECS / row map)."""
    import concourse.dve_ops as dve_ops
    from concourse.dve_ops import DveOp, _CUSTOM_DVE_ROW_BASE
    from concourse.dve_spec import AluOp, Spec, Src0, Src1, scan, lower, _has_src1
    from concourse.dve_uop import DveOpSpec

    for op in dve_ops.OPS:
        if op.name == _SCAN_OP_NAME:
            return op

    def _ref(in0, in1, s0, s1, imm2):
        prod = (in0.astype(np.float32) * in1.astype(np.float32)).reshape(
            in0.shape[0], -1
        )
        return np.cumsum(prod, axis=1).astype(np.float32).reshape(in0.shape)

    spec = Spec(body=scan(AluOp.ADD, Src0 * Src1), reference=_ref)
    row = _CUSTOM_DVE_ROW_BASE + len(dve_ops.OPS)
    assert row < 0x20
    shas = {}
    for ver in ("v3", "v4"):
        s = DveOpSpec(
            name=_SCAN_OP_NAME, opcode=row, uops=lower(spec, ver=ver),
            rd1_en=_has_src1(spec),
        )
        shas[ver] = s.sha(ver)
    op = DveOp(_SCAN_OP_NAME, spec, subdim=False, uops_sha=shas)
    dve_ops.OPS.append(op)
    dve_ops.CUSTOM_DVE_SPECS[_SCAN_OP_NAME] = spec
    dve_ops._SUB_OPCODE_FOR_NAME[_SCAN_OP_NAME] = row
    return op


def _build_body(nc, out_ap, ins, scan_op):
    from contextlib import ExitStack

    import concourse.bass as bass
    import concourse.tile as tile
    from concourse import mybir

    f32 = mybir.dt.float32
    Alu = mybir.AluOpType
    Act = mybir.ActivationFunctionType
    Ax = mybir.AxisListType
    AP = bass.AP

    mem_ap = ins["memory"]   # [B, N, M]
    k_ap = ins["k"]          # [B, M]
    beta_ap = ins["beta"]    # [B, 1]
    pw_ap = ins["prev_w"]    # [B, N]
    g_ap = ins["g"]          # [B, 1]
    s_ap = ins["s"]          # [B, 3]
    gam_ap = ins["gamma"]    # [B, 1]
    rotdn_ap = ins["rotdn"]  # [P, P] constant
    rotup_ap = ins["rotup"]  # [P, P] constant

    def bcast_inner(ap2d, n):
        return AP(ap2d.tensor, ap2d.offset, list(ap2d.ap) + [[0, n]])

    def row1(ap1d):
        return AP(ap1d.tensor, ap1d.offset, [[0, 1]] + list(ap1d.ap))

    with tile.TileContext(nc) as tc, ExitStack() as ctx:
        singles = ctx.enter_context(tc.tile_pool(name="singles", bufs=1))
        mem_pool = ctx.enter_context(tc.tile_pool(name="mem", bufs=8))
        pre_pool = ctx.enter_context(tc.tile_pool(name="pre", bufs=4))
        h_pool = ctx.enter_context(tc.tile_pool(name="hp", bufs=4))
        sm = ctx.enter_context(tc.tile_pool(name="sm", bufs=3))
        ps = ctx.enter_context(tc.tile_pool(name="ps", bufs=2, space="PSUM"))
        ps_big = ctx.enter_context(tc.tile_pool(name="psbig", bufs=1, space="PSUM"))

        # ---- setup: constants, per-batch scalar rows on partition 0 ----
        ones_col = singles.tile([P, 1], f32, tag="ones_col")
        nc.vector.memset(ones_col, 1.0)
        ones_row = singles.tile([1, P], f32, tag="ones_row")
        nc.vector.memset(ones_row, 1.0)

        rotdn_t = singles.tile([P, P], f32, tag="rotdn")
        nc.sync.dma_start(out=rotdn_t, in_=rotdn_ap)
        rotup_t = singles.tile([P, P], f32, tag="rotup")
        nc.sync.dma_start(out=rotup_t, in_=rotup_ap)

        k_row = singles.tile([1, B * M], f32, tag="k_row")
        nc.sync.dma_start(out=k_row, in_=row1(k_ap.rearrange("b m -> (b m)")))
        b_row = singles.tile([1, B], f32, tag="b_row")
        nc.sync.dma_start(out=b_row, in_=row1(beta_ap.rearrange("b one -> (b one)")))
        g_row = singles.tile([1, B], f32, tag="g_row")
        nc.sync.dma_start(out=g_row, in_=row1(g_ap.rearrange("b one -> (b one)")))
        gm_row = singles.tile([1, B], f32, tag="gm_row")
        nc.sync.dma_start(out=gm_row, in_=row1(gam_ap.rearrange("b one -> (b one)")))
        s_row = singles.tile([1, 3 * B], f32, tag="s_row")
        nc.sync.dma_start(out=s_row, in_=row1(s_ap.rearrange("b i -> (b i)")))
        s_perm = s_row.rearrange("p (b i) -> p i b", i=3)
        s_v = [s_perm[:, i, :] for i in range(3)]

        # k broadcast to all partitions: kb[p, b*M+m] = k[b, m]
        k_row2 = singles.tile([1, B * M], f32, tag="setup_scratch")
        nc.vector.tensor_copy(k_row2, k_row)
        kb_psum = ps_big.tile([P, B * M], f32, tag="kb_psum")
        for j in range(0, B * M, 512):
            nc.tensor.matmul(
                kb_psum[:, j : j + 512], ones_row, k_row2[:, j : j + 512],
                start=True, stop=True,
            )
        kb = singles.tile([P, B * M], f32, tag="kb")
        nc.scalar.copy(out=kb, in_=kb_psum)

        # knorm; bk = beta / knorm (reuses the setup scratch row)
        ksq_row = singles.tile([1, B * M], f32, tag="setup_scratch")
        nc.vector.tensor_mul(ksq_row, k_row, k_row)
        ks_row = singles.tile([1, B], f32, tag="ks_row")
        nc.vector.tensor_reduce(
            out=ks_row, in_=ksq_row.rearrange("p (b m) -> p b m", m=M),
            axis=Ax.X, op=Alu.add,
        )
        kn_row = singles.tile([1, B], f32, tag="kn_row")
        nc.scalar.activation(out=kn_row, in_=ks_row, func=Act.Sqrt)
        rk_row = singles.tile([1, B], f32, tag="rk_row")
        nc.vector.reciprocal(out=rk_row, in_=kn_row)
        bk_row = singles.tile([1, B], f32, tag="bk_row")
        nc.vector.tensor_mul(bk_row, b_row, rk_row)

        # omg = 1 - g
        omg_row = singles.tile([1, B], f32, tag="omg_row")
        nc.vector.tensor_scalar(
            out=omg_row, in0=g_row, scalar1=-1.0, scalar2=1.0,
            op0=Alu.mult, op1=Alu.add,
        )

        # broadcast: [bk, omg, s0, s1, s2, gamma] -> [P, 6*B]
        NSC = 6
        asm1 = singles.tile([1, NSC * B], f32, tag="asm1")
        for i, src in enumerate([bk_row, omg_row, s_v[0], s_v[1], s_v[2], gm_row]):
            nc.vector.tensor_copy(asm1[:, i * B : (i + 1) * B], src)
        bc1_ps = ps_big.tile([P, NSC * B], f32, tag="bc1_ps")
        nc.tensor.matmul(bc1_ps, ones_row, asm1, start=True, stop=True)
        BC1 = singles.tile([P, NSC * B], f32, tag="BC1")
        nc.scalar.copy(out=BC1, in_=bc1_ps)
        BK = BC1[:, 0 * B : 1 * B]
        OMG = BC1[:, 1 * B : 2 * B]
        S0 = BC1[:, 2 * B : 3 * B]
        S1 = BC1[:, 3 * B : 4 * B]
        S2 = BC1[:, 4 * B : 5 * B]
        GAM = BC1[:, 5 * B : 6 * B]

        # prev_w big tile [P, B*R]
        pw = singles.tile([P, B * R], f32, tag="pw")
        nc.sync.dma_start(
            out=pw.rearrange("p (b r) -> p b r", r=R),
            in_=pw_ap.rearrange("b (p r) -> p b r", r=R),
        )

        # ---- streamed, per-batch pipelined phase A ----
        BW = R * M  # free elems per batch per partition (2048)
        for c in range(NCHUNK):
            b0 = c * CB
            dotw = sm.tile([P, CB * R], f32, tag="dotw")
            dotw3 = dotw.rearrange("p (b r) -> p b r", r=R)
            ssq = sm.tile([P, CB * R], f32, tag="ssq")
            ssq3 = ssq.rearrange("p (b r) -> p b r", r=R)

            for j in range(CB):
                bj = b0 + j
                mt = mem_pool.tile([P, BW], f32, tag="mt")
                nc.sync.dma_start(
                    out=mt,
                    in_=mem_ap[bj].rearrange("(p r) m -> p (r m)", p=P),
                )
                mt3 = mt.rearrange("p (r m) -> p r m", m=M)

                # fused mult+prefix-scan (custom DVE op). The output AP has a
                # 0-stride inner dim: all 64 writes of window r land on
                # ends[p, r]; DVE writes in stream order, so the final value
                # is the prefix at the window end — a compact [P, R] ends
                # tile with no strided re-read.
                pre = pre_pool.tile([P, R], f32, tag="pre")
                kbj = kb[:, bj * M : (bj + 1) * M]
                kb3 = AP(kbj.tensor, kbj.offset, [kbj.ap[0], [0, R], [1, M]])
                nc.vector._custom_dve(
                    scan_op,
                    out=AP(pre.tensor, pre.offset, [pre.ap[0], [1, R], [0, M]]),
                    in0=mt3, in1=kb3,
                )

                # dot rows by differencing consecutive window-end prefixes
                nc.vector.tensor_copy(dotw3[:, j, 0:1], pre[:, 0:1])
                nc.vector.tensor_tensor(
                    out=dotw3[:, j, 1:R], in0=pre[:, 1:R], in1=pre[:, 0 : R - 1],
                    op=Alu.subtract,
                )

                # ssq: ACT square in place, GpSimd fold 64->32->16, DVE reduce
                nc.scalar.square(out=mt, in_=mt)
                hb = h_pool.tile([P, R * 48], f32, tag="hb")
                h1 = hb[:, 0 : R * 32].rearrange("p (r m) -> p r m", m=32)
                nc.gpsimd.tensor_tensor(
                    out=h1, in0=mt3[:, :, 0:32], in1=mt3[:, :, 32:64],
                    op=Alu.add,
                )
                h2 = hb[:, R * 32 : R * 48].rearrange("p (r m) -> p r m", m=16)
                nc.gpsimd.tensor_tensor(
                    out=h2, in0=h1[:, :, 0:16], in1=h1[:, :, 16:32],
                    op=Alu.add,
                )
                nc.vector.tensor_reduce(
                    out=ssq3[:, j, :], in_=h2, axis=Ax.X, op=Alu.add,
                )

            # ---- phase B for this chunk ----
            BKc = BK[:, b0 : b0 + CB]
            OMGc = OMG[:, b0 : b0 + CB]
            S0c = S0[:, b0 : b0 + CB]
            S1c = S1[:, b0 : b0 + CB]
            S2c = S2[:, b0 : b0 + CB]
            GAMc = GAM[:, b0 : b0 + CB]

            def v3(t):
                return t.rearrange("p (b r) -> p b r", r=R)

            # rstd = 1/sqrt(ssq); a = dot * rstd * bk
            mn = sm.tile([P, CB * R], f32, tag="mn")
            nc.scalar.activation(out=mn, in_=ssq, func=Act.Sqrt)
            rstd = sm.tile([P, CB * R], f32, tag="rstd")
            nc.vector.reciprocal_approx_fast(out=rstd, in_=mn)
            nc.vector.tensor_mul(dotw, dotw, rstd)
            nc.vector.tensor_mul(dotw3, dotw3, bcast_inner(BKc, R))

            # e = exp(a); per-batch denom; gd = g/denom
            mmall = ps.tile([P, 6 * CB], f32, tag="mmall")
            den_ps = mmall[0:1, 0 * CB : 1 * CB]
            gd_ps = mmall[:, 1 * CB : 2 * CB]
            dn_ps = mmall[:, 2 * CB : 3 * CB]
            up_ps = mmall[:, 3 * CB : 4 * CB]
            d2_ps = mmall[0:1, 4 * CB : 5 * CB]
            rd2_ps = mmall[:, 5 * CB : 6 * CB]

            e = sm.tile([P, CB * R], f32, tag="e")
            nc.scalar.activation(out=e, in_=dotw, func=Act.Exp)
            cs = sm.tile([P, CB], f32, tag="cs")
            nc.vector.tensor_reduce(out=cs, in_=v3(e), axis=Ax.X, op=Alu.add)
            nc.tensor.matmul(den_ps, ones_col, cs, start=True, stop=True)
            rden = sm.tile([1, CB], f32, tag="rden")
            nc.vector.reciprocal(out=rden, in_=den_ps)
            gd = sm.tile([1, CB], f32, tag="gd")
            nc.vector.tensor_mul(gd, rden, g_row[:, b0 : b0 + CB])
            nc.tensor.matmul(gd_ps, ones_row, gd, start=True, stop=True)

            # wg = e*gd + pw*omg  (GpSimd cannot read PSUM: stage gd in SBUF)
            gd_sb = sm.tile([P, CB], f32, tag="gd_sb")
            nc.scalar.copy(out=gd_sb, in_=gd_ps)
            pwc = sm.tile([P, CB * R], f32, tag="pwc")
            pwv = pw[:, b0 * R : (b0 + CB) * R]
            nc.gpsimd.tensor_tensor(
                out=v3(e), in0=v3(e), in1=bcast_inner(gd_sb, R), op=Alu.mult
            )
            nc.gpsimd.tensor_tensor(
                out=v3(pwc), in0=v3(pwv), in1=bcast_inner(OMGc, R), op=Alu.mult
            )
            nc.gpsimd.tensor_tensor(out=e, in0=e, in1=pwc, op=Alu.add)

            # circular 3-tap shift
            ws = sm.tile([P, CB * R], f32, tag="ws")
            ta = sm.tile([P, CB * R], f32, tag="ta")
            tb = sm.tile([P, CB * R], f32, tag="tb")
            wg3, ws3, ta3, tb3 = v3(e), v3(ws), v3(ta), v3(tb)
            nc.vector.tensor_mul(ws3, wg3, bcast_inner(S1c, R))
            nc.vector.tensor_mul(ta3, wg3, bcast_inner(S0c, R))
            nc.gpsimd.tensor_tensor(
                out=tb3, in0=wg3, in1=bcast_inner(S2c, R), op=Alu.mult
            )
            nc.vector.tensor_add(
                out=ws3[:, :, 1:R], in0=ws3[:, :, 1:R], in1=ta3[:, :, 0 : R - 1]
            )
            nc.vector.tensor_add(
                out=ws3[:, :, 0 : R - 1], in0=ws3[:, :, 0 : R - 1],
                in1=tb3[:, :, 1:R],
            )
            # partition carries via rotation matmuls
            nc.tensor.matmul(
                dn_ps, rotdn_t,
                AP(ta.tensor, ta.offset + R - 1, [ta.ap[0], [R, CB]]),
                start=True, stop=True,
            )
            nc.tensor.matmul(
                up_ps, rotup_t,
                AP(tb.tensor, tb.offset, [tb.ap[0], [R, CB]]),
                start=True, stop=True,
            )
            nc.vector.tensor_add(
                out=ws3[:, :, 0:1], in0=ws3[:, :, 0:1],
                in1=bcast_inner(dn_ps, 1),
            )
            nc.vector.tensor_add(
                out=ws3[:, :, R - 1 : R], in0=ws3[:, :, R - 1 : R],
                in1=bcast_inner(up_ps, 1),
            )

            # w_pow = ws ** gamma; normalize
            nc.scalar.activation(out=ws, in_=ws, func=Act.Ln)
            nc.gpsimd.tensor_tensor(
                out=ws3, in0=ws3, in1=bcast_inner(GAMc, R), op=Alu.mult
            )
            nc.scalar.activation(out=ws, in_=ws, func=Act.Exp)
            cs2 = sm.tile([P, CB], f32, tag="cs2")
            nc.vector.tensor_reduce(out=cs2, in_=ws3, axis=Ax.X, op=Alu.add)
            nc.tensor.matmul(d2_ps, ones_col, cs2, start=True, stop=True)
            d2r = sm.tile([1, CB], f32, tag="d2r")
            nc.vector.tensor_scalar_add(out=d2r, in0=d2_ps, scalar1=1e-16)
            rd2 = sm.tile([1, CB], f32, tag="rd2")
            nc.vector.reciprocal(out=rd2, in_=d2r)
            nc.tensor.matmul(rd2_ps, ones_row, rd2, start=True, stop=True)
            rd2_sb = sm.tile([P, CB], f32, tag="rd2_sb")
            nc.scalar.copy(out=rd2_sb, in_=rd2_ps)
            outsb = sm.tile([P, CB * R], f32, tag="outsb")
            nc.gpsimd.tensor_tensor(
                out=v3(outsb), in0=ws3, in1=bcast_inner(rd2_sb, R), op=Alu.mult
            )

            nc.sync.dma_start(
                out=out_ap[b0 : b0 + CB].rearrange("b (p r) -> p b r", r=R),
                in_=outsb.rearrange("p (b r) -> p b r", r=R),
            )


def _get_nc():
    if "nc" in _NC_CACHE:
        return _NC_CACHE["nc"]
    scan_op = _register_scan_op()
    from concourse import bacc, mybir

    f32 = mybir.dt.float32
    nc = bacc.Bacc("TRN2", debug=False, num_devices=NCORES)
    ins = {
        "memory": nc.dram_tensor("memory", [B, N, M], f32, kind="ExternalInput").ap(),
        "k": nc.dram_tensor("k", [B, M], f32, kind="ExternalInput").ap(),
        "beta": nc.dram_tensor("beta", [B, 1], f32, kind="ExternalInput").ap(),
        "prev_w": nc.dram_tensor("prev_w", [B, N], f32, kind="ExternalInput").ap(),
        "g": nc.dram_tensor("g", [B, 1], f32, kind="ExternalInput").ap(),
        "s": nc.dram_tensor("s", [B, 3], f32, kind="ExternalInput").ap(),
        "gamma": nc.dram_tensor("gamma", [B, 1], f32, kind="ExternalInput").ap(),
        "rotdn": nc.dram_tensor("rotdn", [P, P], f32, kind="ExternalInput").ap(),
        "rotup": nc.dram_tensor("rotup", [P, P], f32, kind="ExternalInput").ap(),
    }
    out_ap = nc.dram_tensor("out", [B, N], f32, kind="ExternalOutput").ap()
    _build_body(nc, out_ap, ins, scan_op)
    nc.finalize()
    _NC_CACHE["nc"] = nc
    return nc


def _rot_mats():
    idx = np.arange(P)
    rotdn = np.zeros((P, P), dtype=np.float32)
    rotdn[(idx - 1) % P, idx] = 1.0   # out[p] = x[(p-1) mod P]
    rotup = np.zeros((P, P), dtype=np.float32)
    rotup[(idx + 1) % P, idx] = 1.0   # out[p] = x[(p+1) mod P]
    return rotdn, rotup


def _shard_inputs(inputs):
    arrs = {
        name: np.ascontiguousarray(np.asarray(inputs[name], dtype=np.float32))
        for name in ("memory", "k", "beta", "prev_w", "g", "s", "gamma")
    }
    rotdn, rotup = _rot_mats()
    in_maps = []
    for c in range(NCORES):
        sl = slice(c * B, (c + 1) * B)
        m = {name: np.ascontiguousarray(a[sl]) for name, a in arrs.items()}
        m["rotdn"] = rotdn
        m["rotup"] = rotup
        in_maps.append(m)
    return in_maps


def run(inputs, trace=False):
    from concourse.bass_utils import run_bass_kernel_spmd

    nc = _get_nc()
    in_maps = _shard_inputs(inputs)
    res = run_bass_kernel_spmd(
        nc, in_maps, core_ids=list(range(NCORES)), trace=trace,
        **({"trace_cores": [0]} if trace else {}),
    )
    out = np.concatenate([r["out"] for r in res.results], axis=0)
    return out, res


def kernel(**inputs):
    out, _ = run(inputs, trace=False)
    return out
